# revision 37
# baseline (speedup 1.0000x reference)
"""Trainium2 Bass kernel for nn_AttentionHeteroRGCNLayer.

Math: softmax of a length-1 vector is 1.0, so the per-relation attention
weights are w = softmax([1,1,1]) = 1/3 each (computed generally anyway).
h = feat @ Wc with Wc = sum_r w_r W_r, and aggregation is linear, so the
layer is out = LN(relu(agg_feat @ Wc)) with per-edge weight
w_e = w_r / max(deg_r[dst_e], 1) folded into a one-hot scatter matrix:
    agg_feat[dst] = sum_e w_e * feat[src_e]

Distribution: dst-range sharding across 8 cores (6400 dst rows each, N padded
to 51200); the bf16 feat table is replicated to every core (no collectives).
Host buckets edges by dst into 256-dst "supers" (dense streams, split lo/hi
for dma_gather's int16 indices); the device gathers rows with dma_gather
(single-packet descriptor streams, calls balanced across the 4 SWDGE queues),
builds 64-wide one-hot scatter blocks per (tile, window) pair with broadcast
is_equal ops in a c-major layout (packed last dims keep the DVE in its fast
mode), and aggregates with bf16 matmuls into PSUM 64-row halves. Per 128-dst
block, Wc is applied via two PE transposes + two matmuls, then ReLU; LN
statistics accumulate via activation accumulators into [128, nblk] tiles and
are reduced in batches on the vector engine, with the final normalization a
single scalar-engine activation (per-partition scale/bias) per block.
"""
import os
import numpy as np
import ml_dtypes

import concourse.bacc as bacc
import concourse.bass as bass
import concourse.mybir as mybir
import concourse.tile as tile
from concourse.bass_utils import run_bass_kernel_spmd

BF16 = mybir.dt.bfloat16
F32 = mybir.dt.float32
NP_BF16 = np.dtype(ml_dtypes.bfloat16)

N = 50000
D = 256
P = 128
WIN = 64                 # one-hot window width
NC = 8
NPAD = 51200
SUPER_DST = 256                      # dsts per super
LO_SPLIT = 32768
MAX_TILES_PER_CALL = 17
LN_EPS = 1e-5
NQ = 4
GROUP_BLOCKS = 50        # LN stat batch size (128-dst blocks): one end flush

SINGLE_PACKET = os.environ.get("K_SINGLE_PACKET", "0") == "1"
CMAJOR = os.environ.get("K_CMAJOR", "1") == "1"
NOTRIM = os.environ.get("K_NOTRIM", "0") == "1"


def _rows_per_core():
    return NPAD // NC


def _supers_per_core():
    return _rows_per_core() // SUPER_DST


def _kwin():
    return SUPER_DST // WIN


def _bf16(x):
    return np.asarray(x, dtype=np.float32).astype(NP_BF16)


def _softmax(v):
    e = np.exp(v - v.max())
    return e / e.sum()


def _even_chunks(total, n):
    base, rem = divmod(total, n)
    out = []
    ofs = 0
    for i in range(n):
        c = base + (1 if i < rem else 0)
        out.append((ofs, c))
        ofs += c
    return out


def _plan_calls(tl, th):
    """Split a super's lo/hi tile streams into exactly NQ (or 2*NQ) gather
    calls. The tile framework assigns SWDGE sem lanes round-robin (8 lanes),
    so queue_num must follow the global issue cycle — callers map call i of
    super s to queue (i + off) % NQ with a fixed per-super call count.
    Returns [(kind, ofs, cnt), ...]."""
    for ncalls in (NQ, 2 * NQ):
        best = None
        for nlo in range(1, ncalls):
            nhi = ncalls - nlo
            if -(-tl // nlo) > MAX_TILES_PER_CALL or -(-th // nhi) > MAX_TILES_PER_CALL:
                continue
            if nlo > tl or nhi > th:
                continue
            chunks = ([("lo", o, c) for o, c in _even_chunks(tl, nlo)]
                      + [("hi", o, c) for o, c in _even_chunks(th, nhi)])
            sizes = [c for _, _, c in chunks]
            key = (max(sizes), sum(c * c for c in sizes))
            if best is None or key < best[0]:
                best = (key, chunks)
        if best is not None:
            return best[1]
    raise AssertionError(f"cannot plan calls for tl={tl} th={th}")


def _host_prep(W0, W1, W2, a0, a1, a2, srcs, dsts):
    supers_per_core = _supers_per_core()
    kwin = _kwin()
    w3 = _softmax(np.concatenate([_softmax(np.asarray(a, np.float64).ravel())
                                  for a in (a0, a1, a2)]))
    Wc = (w3[0] * np.asarray(W0, np.float32) + w3[1] * np.asarray(W1, np.float32)
          + w3[2] * np.asarray(W2, np.float32)).astype(np.float32)

    src_all, dst_all, wgt_all = [], [], []
    for r in range(3):
        s = np.asarray(srcs[r], np.int64)
        d = np.asarray(dsts[r], np.int64)
        deg = np.bincount(d, minlength=N)
        w_e = (w3[r] / np.maximum(deg, 1.0)[d]).astype(np.float32)
        src_all.append(s); dst_all.append(d); wgt_all.append(w_e)
    src_all = np.concatenate(src_all)
    dst_all = np.concatenate(dst_all)
    wgt_all = np.concatenate(wgt_all)

    order = np.argsort(dst_all, kind="stable")
    s_s, d_s, w_s = src_all[order], dst_all[order], wgt_all[order]

    # per (core, super) lo/hi streams: (src, dst_rel[0..256), wgt)
    gsup = d_s // SUPER_DST
    sup_counts = np.bincount(gsup, minlength=NC * supers_per_core)
    sup_start = np.zeros(NC * supers_per_core + 1, np.int64)
    np.cumsum(sup_counts, out=sup_start[1:])

    streams = {}     # (c, s, 'lo'/'hi') -> (src_idx, dst_rel, wgt)
    n_lo = np.zeros((NC, supers_per_core), np.int64)
    n_hi = np.zeros((NC, supers_per_core), np.int64)
    for g in range(NC * supers_per_core):
        c, s = g // supers_per_core, g % supers_per_core
        a, b = sup_start[g], sup_start[g + 1]
        sl_s, sl_d, sl_w = s_s[a:b], d_s[a:b], w_s[a:b]
        rel = sl_d - g * SUPER_DST
        m = sl_s < LO_SPLIT
        streams[(c, s, "lo")] = (sl_s[m], rel[m], sl_w[m])
        streams[(c, s, "hi")] = (sl_s[~m] - LO_SPLIT, rel[~m], sl_w[~m])
        n_lo[c, s] = int(m.sum())
        n_hi[c, s] = int((~m).sum())

    T_lo = np.maximum(1, -(-n_lo.max(axis=0) // P))
    T_hi = np.maximum(1, -(-n_hi.max(axis=0) // P))

    n_lo_max = n_lo.max(axis=0)
    n_hi_max = n_hi.max(axis=0)

    schedule = []
    total_tiles = 0
    total_pairs = 0
    for s in range(supers_per_core):
        tl, th = int(T_lo[s]), int(T_hi[s])
        ntile = tl + th
        nmax = {"lo": tl * P, "hi": th * P} if NOTRIM else \
            {"lo": int(n_lo_max[s]), "hi": int(n_hi_max[s])}
        calls = [(kind, ofs, cnt,
                  int(np.clip(nmax[kind] - ofs * P, 0, cnt * P)))
                 for kind, ofs, cnt in _plan_calls(tl, th)]
        # rotate emission so the big chunks cycle through the queues
        r = s % len(calls)
        calls = calls[r:] + calls[:r]
        # pair schedule: for each tile, cross-core k-window range
        tile_kmin = np.full(ntile, kwin, np.int64)
        tile_kmax = np.full(ntile, -1, np.int64)
        for c in range(NC):
            for kind, tbase, tcnt in (("lo", 0, tl), ("hi", tl, th)):
                rel = streams[(c, s, kind)][1]
                n = len(rel)
                if n == 0:
                    continue
                kk = rel // WIN
                for t in range(min(tcnt, -(-n // P))):
                    seg = kk[t * P:(t + 1) * P]
                    tile_kmin[tbase + t] = min(tile_kmin[tbase + t], int(seg.min()))
                    tile_kmax[tbase + t] = max(tile_kmax[tbase + t], int(seg.max()))
        pairs = []          # (tile, k)
        for t in range(ntile):
            if tile_kmax[t] < 0:
                continue
            for k in range(int(tile_kmin[t]), int(tile_kmax[t]) + 1):
                pairs.append((t, k))
        covered = {k for _, k in pairs}
        for k in range(kwin):
            if k not in covered:
                pairs.append((0, k))
        # group by k for clean PSUM accumulation chains, tiles in order
        by_k = {k: [] for k in range(kwin)}
        for t, k in pairs:
            by_k[k].append(t)
        pair_sched = []     # (k, tile, paircol)
        paircol = 0
        for k in range(kwin):
            for t in sorted(by_k[k]):
                pair_sched.append((k, t, total_pairs + paircol))
                paircol += 1
        npairs = paircol
        schedule.append(dict(
            tile_base=total_tiles, pair_base=total_pairs,
            tl=tl, th=th, ntile=ntile, calls=calls, nmax=nmax,
            pair_sched=pair_sched, npairs=npairs,
        ))
        total_tiles += ntile
        total_pairs += npairs

    # host metadata arrays
    idx16 = np.zeros((NC, P, total_tiles * 8), np.int16)
    dlmat = np.full((NC, P, total_pairs), -1000.0, np.float32)
    wgmat = np.zeros((NC, P, total_pairs), np.float32)

    for c in range(NC):
        for s in range(supers_per_core):
            sc = schedule[s]
            tl, th = sc["tl"], sc["th"]
            relcap = np.full((sc["ntile"], P), -1000.0, np.float32)
            wgtcap = np.zeros((sc["ntile"], P), np.float32)
            for kind, tbase, tcnt in (("lo", 0, tl), ("hi", tl, th)):
                es, rel, ew = streams[(c, s, kind)]
                n = len(es)
                cap = tcnt * P
                nmax = cap if NOTRIM else \
                    int((n_lo_max if kind == "lo" else n_hi_max)[s])
                # pad with row 0 up to the cross-core max valid count (the
                # per-call num_idxs_reg, shared by the SPMD program), -1 after
                # (ucode skips trailing negatives)
                sidx = np.zeros(cap, np.int64)
                sidx[:n] = es
                sidx[nmax:] = -1
                rl = np.full(cap, -1000.0, np.float32)
                rl[:n] = rel.astype(np.float32)
                wv = np.zeros(cap, np.float32)
                wv[:n] = ew
                relcap[tbase:tbase + tcnt] = rl.reshape(tcnt, P)
                wgtcap[tbase:tbase + tcnt] = wv.reshape(tcnt, P)
                iw = sidx.reshape(tcnt, 8, 16).transpose(0, 2, 1)   # [t,16,8]
                iw = np.tile(iw, (1, 8, 1))                          # [t,128,8]
                tb = sc["tile_base"] + tbase
                idx16[c, :, tb * 8:(tb + tcnt) * 8] = (
                    iw.transpose(1, 0, 2).reshape(P, tcnt * 8).astype(np.int16))
            for k, t, pc in sc["pair_sched"]:
                dlmat[c, :, pc] = relcap[t] - k * WIN
                wgmat[c, :, pc] = wgtcap[t]

    return dict(Wc=Wc, schedule=schedule, total_tiles=total_tiles,
                total_pairs=total_pairs, idx16=idx16, dlmat=dlmat, wgmat=wgmat)


def _build_nc(schedule, total_tiles, total_pairs, apply_affine):
    supers_per_core = _supers_per_core()
    rows_per_core = _rows_per_core()
    kwin = _kwin()
    nblk = 2 * supers_per_core
    nc = bacc.Bacc(None, target_bir_lowering=False, num_swdge_queues=NQ)
    tab_lo = nc.declare_dram_parameter("tab_lo", [LO_SPLIT, D], BF16, isOutput=False)
    tab_hi = nc.declare_dram_parameter("tab_hi", [N - LO_SPLIT, D], BF16, isOutput=False)
    idx_d = nc.declare_dram_parameter("idx", [P, total_tiles * 8], mybir.dt.int16, isOutput=False)
    dl_d = nc.declare_dram_parameter("dl", [P, total_pairs], BF16, isOutput=False)
    wg_d = nc.declare_dram_parameter("wg", [P, total_pairs], BF16, isOutput=False)
    wc_d = nc.declare_dram_parameter("wc", [P, 2 * D], BF16, isOutput=False)
    max_np = max(sc["npairs"] for sc in schedule)
    iota_cols = WIN * max_np if CMAJOR else WIN
    # cst: iota (c-major [WIN, max_np] or plain [WIN]) | identity
    cst_d = nc.declare_dram_parameter("cst", [P, iota_cols + P], BF16, isOutput=False)
    gb_d = nc.declare_dram_parameter("gb", [P, 2 * D], F32, isOutput=False)
    out_d = nc.declare_dram_parameter("out", [rows_per_core, D], BF16, isOutput=True)

    max_tl = max(sc["tl"] for sc in schedule)
    max_th = max(sc["th"] for sc in schedule)
    qrot = [0]

    with tile.TileContext(nc) as tc:
        with (
            tc.tile_pool(name="meta", bufs=1) as meta_pool,
            tc.tile_pool(name="xlo", bufs=4) as xlo_pool,
            tc.tile_pool(name="xhi", bufs=4) as xhi_pool,
            tc.tile_pool(name="bmat0", bufs=1) as b0_pool,
            tc.tile_pool(name="bmat", bufs=2) as b_pool,
            tc.tile_pool(name="ev", bufs=3) as ev_pool,
            tc.tile_pool(name="yo", bufs=3) as y_pool,
            tc.tile_pool(name="xst", bufs=1) as xst_pool,
            tc.tile_pool(name="stat", bufs=1) as stat_pool,
            tc.tile_pool(name="psA", bufs=3, space="PSUM") as psA,
            tc.tile_pool(name="psB", bufs=2, space="PSUM") as psB,
            tc.tile_pool(name="psC", bufs=2, space="PSUM") as psC,
        ):
            idx_sb = meta_pool.tile([P, total_tiles * 8], mybir.dt.int16)
            # chunked so the first supers' gathers don't wait on the full load
            idx_chunk = -(-total_tiles // 5) * 8
            for j in range(0, total_tiles * 8, idx_chunk):
                e = min(j + idx_chunk, total_tiles * 8)
                nc.sync.dma_start(out=idx_sb[:, j:e], in_=idx_d[:, j:e])
            mrow = meta_pool.tile([P, 2 * total_pairs + 2 * D + iota_cols + P], BF16)
            nc.sync.dma_start(out=mrow[:, :total_pairs], in_=dl_d[:])
            nc.sync.dma_start(out=mrow[:, total_pairs:2 * total_pairs], in_=wg_d[:])
            nc.sync.dma_start(out=mrow[:, 2 * total_pairs:2 * total_pairs + 2 * D], in_=wc_d[:])
            nc.sync.dma_start(out=mrow[:, 2 * total_pairs + 2 * D:], in_=cst_d[:])
            dl_sb = mrow[:, 0:total_pairs]
            wg_sb = mrow[:, total_pairs:2 * total_pairs]
            wc_sb = mrow[:, 2 * total_pairs:2 * total_pairs + 2 * D]
            iota_sb = mrow[:, 2 * total_pairs + 2 * D:2 * total_pairs + 2 * D + iota_cols]
            ident_sb = mrow[:, 2 * total_pairs + 2 * D + iota_cols:]
            gb_sb = meta_pool.tile([P, 2 * D], F32)
            nc.sync.dma_start(out=gb_sb[:], in_=gb_d[:])
            gamma_sb = gb_sb[:, 0:D]
            beta_sb = gb_sb[:, D:2 * D]



            eps_tile = meta_pool.tile([P, 1], F32)
            nc.vector.memset(eps_tile[:], LN_EPS)
            eps_col = eps_tile[:]

            # per-group LN staging/stat tiles (separate tiles, not slices of
            # one big tile: dependency tracking is tile-granular, so a shared
            # tile would make group reads wait on ALL writes issued so far,
            # head-of-line blocking the engine queues)
            GB = GROUP_BLOCKS
            group_tiles = {}

            def group_of(w):
                g = w // GB
                if g not in group_tiles:
                    xst_g = xst_pool.tile([P, GB * D], BF16, tag="xst")
                    s1_g = stat_pool.tile([P, GB], F32, tag="s1")
                    s2_g = stat_pool.tile([P, GB], F32, tag="s2")
                    st_g = stat_pool.tile([P, 5 * GB], F32, tag="st")
                    group_tiles[g] = dict(xst=xst_g, s1=s1_g, s2=s2_g, st=st_g)
                return group_tiles[g]

            def flush_group(g0, g1):
                n = g1 - g0
                if n <= 0:
                    return
                gt = group_tiles[g0 // GB]
                s1, s2 = gt["s1"][:, :n], gt["s2"][:, :n]
                st = gt["st"]
                mu = st[:, 0 * GB:0 * GB + n]
                mm = st[:, 1 * GB:1 * GB + n]
                var = st[:, 2 * GB:2 * GB + n]
                sd = st[:, 3 * GB:3 * GB + n]
                rstd = st[:, 4 * GB:4 * GB + n]
                nmb = mm  # reuse mm slot for -mu*rstd (mm dead after var)
                nc.vector.tensor_scalar(out=mu, in0=s1,
                                        scalar1=1.0 / D, scalar2=None,
                                        op0=mybir.AluOpType.mult)
                nc.vector.tensor_tensor(out=mm, in0=mu, in1=mu,
                                        op=mybir.AluOpType.mult)
                nc.vector.scalar_tensor_tensor(out=var, in0=s2,
                                               scalar=1.0 / D, in1=mm,
                                               op0=mybir.AluOpType.mult,
                                               op1=mybir.AluOpType.subtract)
                nc.vector.tensor_scalar(out=var, in0=var,
                                        scalar1=0.0, scalar2=None,
                                        op0=mybir.AluOpType.max)
                nc.scalar.activation(out=sd, in_=var,
                                     func=mybir.ActivationFunctionType.Sqrt,
                                     bias=eps_col)
                nc.vector.reciprocal(out=rstd, in_=sd)
                nc.vector.scalar_tensor_tensor(out=nmb, in0=mu,
                                               scalar=-1.0, in1=rstd,
                                               op0=mybir.AluOpType.mult,
                                               op1=mybir.AluOpType.mult)
                for w in range(g0, g1):
                    j = w - g0
                    yout = y_pool.tile([P, D], BF16, tag="y")
                    nc.scalar.activation(out=yout[:],
                                         in_=gt["xst"][:, j * D:(j + 1) * D],
                                         func=mybir.ActivationFunctionType.Identity,
                                         scale=rstd[:, j:j + 1], bias=nmb[:, j:j + 1])
                    if apply_affine:
                        y2 = y_pool.tile([P, D], F32, tag="y2")
                        nc.vector.tensor_tensor(out=y2[:], in0=yout[:], in1=gamma_sb,
                                                op=mybir.AluOpType.mult)
                        y3 = y_pool.tile([P, D], BF16, tag="y3")
                        nc.vector.tensor_tensor(out=y3[:], in0=y2[:], in1=beta_sb,
                                                op=mybir.AluOpType.add)
                        yfin = y3
                    else:
                        yfin = yout
                    nc.sync.dma_start(out=out_d[w * P:(w + 1) * P, :], in_=yfin[:])

            done_blocks = 0
            for s in range(supers_per_core):
                sc = schedule[s]
                base = sc["tile_base"]
                pbase = sc["pair_base"]
                tl, th, npairs = sc["tl"], sc["th"], sc["npairs"]
                xlo = xlo_pool.tile([P, max_tl * D], BF16, tag="xlo")
                xhi = xhi_pool.tile([P, max_th * D], BF16, tag="xhi")
                # zero the pad tail the gather will skip (trailing -1 idxs):
                # the one-hot columns there are zero, but 0 * NaN from stale
                # slots would still poison the PSUM accumulate
                for kind, x_t, tcnt in (("lo", xlo, tl), ("hi", xhi, th)):
                    pt = sc["nmax"][kind] // P
                    if pt < tcnt:
                        nc.vector.memset(x_t[:, pt * D:tcnt * D], 0.0)
                for kind, ofs, cnt, reg in sc["calls"]:
                    x_t, tab, tofs = (xlo, tab_lo, base + ofs) if kind == "lo" \
                        else (xhi, tab_hi, base + tl + ofs)
                    ni = cnt * P
                    nc.gpsimd.dma_gather(
                        out_ap=x_t[:, ofs * D:(ofs + cnt) * D].rearrange(
                            "p (t e) -> p t e", e=D),
                        in_ap=tab[:],
                        idxs_ap=idx_sb[:, tofs * 8:(tofs + cnt) * 8],
                        num_idxs=ni, num_idxs_reg=reg, elem_size=D,
                        single_packet=SINGLE_PACKET,
                        queue_num=qrot[0] % NQ,
                    )
                    qrot[0] += 1

                if CMAJOR:
                    bmat0 = b0_pool.tile([P, max_np * WIN], BF16, tag="b0")
                    bmat = b_pool.tile([P, max_np * WIN], BF16, tag="b")
                    b0view = bmat0[:, :npairs * WIN].rearrange(
                        "p (c t) -> p c t", t=npairs)
                    bview = bmat[:, :npairs * WIN].rearrange(
                        "p (c t) -> p c t", t=npairs)
                    iview = iota_sb.rearrange(
                        "p (c t) -> p c t", t=max_np)[:, :, 0:npairs]
                    nc.vector.tensor_tensor(
                        out=b0view, in0=iview,
                        in1=dl_sb[:, pbase:pbase + npairs].unsqueeze(1).to_broadcast(
                            [P, WIN, npairs]),
                        op=mybir.AluOpType.is_equal,
                    )
                    nc.vector.tensor_tensor(
                        out=bview, in0=b0view,
                        in1=wg_sb[:, pbase:pbase + npairs].unsqueeze(1).to_broadcast(
                            [P, WIN, npairs]),
                        op=mybir.AluOpType.mult,
                    )

                    def lhsT_of(pc_rel):
                        return bmat[:, :npairs * WIN].rearrange(
                            "p (c t) -> p c t", t=npairs)[:, :, pc_rel:pc_rel + 1]
                else:
                    bmat = b_pool.tile([P, max_np * WIN], BF16, tag="b")
                    nc.vector.tensor_tensor(
                        out=bmat[:, :npairs * WIN].rearrange("p (t c) -> p t c", c=WIN),
                        in0=iota_sb.unsqueeze(1).to_broadcast([P, npairs, WIN]),
                        in1=dl_sb[:, pbase:pbase + npairs].unsqueeze(2).to_broadcast(
                            [P, npairs, WIN]),
                        op=mybir.AluOpType.is_equal,
                    )
                    nc.vector.tensor_tensor(
                        out=bmat[:, :npairs * WIN].rearrange("p (t c) -> p t c", c=WIN),
                        in0=bmat[:, :npairs * WIN].rearrange("p (t c) -> p t c", c=WIN),
                        in1=wg_sb[:, pbase:pbase + npairs].unsqueeze(2).to_broadcast(
                            [P, npairs, WIN]),
                        op=mybir.AluOpType.mult,
                    )

                    def lhsT_of(pc_rel):
                        return bmat[:, pc_rel * WIN:(pc_rel + 1) * WIN]

                # aggregation into PSUM 64-row halves; pair_sched grouped by k
                aggb = psA.tile([P, 2 * D], F32, tag="agg")
                aggs = [aggb[:, 0:D], aggb[:, D:2 * D]]
                ksched = {}
                for k, t, pc in sc["pair_sched"]:
                    ksched.setdefault(k, []).append((t, pc))
                for k in range(kwin):
                    plist = ksched[k]
                    half = k % 2
                    agg = aggs[k // 2]
                    for i, (t, pc) in enumerate(plist):
                        xsrc = xlo if t < tl else xhi
                        xoff = t if t < tl else t - tl
                        nc.tensor.matmul(
                            out=agg[half * WIN:(half + 1) * WIN, :],
                            lhsT=lhsT_of(pc - pbase),
                            rhs=xsrc[:, xoff * D:(xoff + 1) * D],
                            start=(i == 0), stop=(i == len(plist) - 1),
                        )

                for b in range(2):
                    w = 2 * s + b          # 128-dst evac block
                    agg = aggs[b]
                    aggS = ev_pool.tile([P, D], BF16, tag="aggS")
                    nc.scalar.activation(out=aggS[:], in_=agg[:],
                                         func=mybir.ActivationFunctionType.Copy)
                    trp = psB.tile([P, D], BF16, tag="trp")
                    nc.tensor.transpose(out=trp[:, 0:P], in_=aggS[:, 0:P], identity=ident_sb)
                    nc.tensor.transpose(out=trp[:, P:D], in_=aggS[:, P:D], identity=ident_sb)
                    aggT = ev_pool.tile([P, D], BF16, tag="aggT")
                    nc.scalar.activation(out=aggT[:], in_=trp[:],
                                         func=mybir.ActivationFunctionType.Copy)
                    out2 = psC.tile([P, D], F32, tag="out2")
                    nc.tensor.matmul(out=out2[:], lhsT=aggT[:, 0:P],
                                     rhs=wc_sb[:, 0:D], start=True, stop=False)
                    nc.tensor.matmul(out=out2[:], lhsT=aggT[:, P:D],
                                     rhs=wc_sb[:, D:2 * D], start=False, stop=True)
                    # ReLU into bf16 staging, accumulate sum into s1
                    gt = group_of(w)
                    j = w % GB
                    nc.scalar.activation(out=gt["xst"][:, j * D:(j + 1) * D],
                                         in_=out2[:],
                                         func=mybir.ActivationFunctionType.Relu,
                                         accum_out=gt["s1"][:, j:j + 1])
                    xsq = ev_pool.tile([P, D], BF16, tag="xsq")
                    nc.scalar.activation(out=xsq[:],
                                         in_=gt["xst"][:, j * D:(j + 1) * D],
                                         func=mybir.ActivationFunctionType.Square,
                                         accum_out=gt["s2"][:, j:j + 1])

                # flush LN groups with a 2-super delay so the stat reads never
                # head-of-line block the vector queue behind pending
                # accumulator writes (stalls the whole gather pipeline)
                blocks_ready = 2 * (s - 1)
                while blocks_ready - done_blocks >= GROUP_BLOCKS:
                    flush_group(done_blocks, done_blocks + GROUP_BLOCKS)
                    done_blocks += GROUP_BLOCKS
            while done_blocks < nblk:
                g1 = min(done_blocks + GROUP_BLOCKS, nblk)
                flush_group(done_blocks, g1)
                done_blocks = g1
    nc.compile()
    return nc


def _host_tensors(prep, gamma, beta):
    max_np = max(sc["npairs"] for sc in prep["schedule"])
    wc_host = np.zeros((P, 2 * D), np.float32)
    wc_host[:, 0:D] = prep["Wc"][0:P, :]
    wc_host[:, D:2 * D] = prep["Wc"][P:D, :]
    iota_cols = WIN * max_np if CMAJOR else WIN
    cst_host = np.zeros((P, iota_cols + P), np.float32)
    if CMAJOR:
        iota = np.repeat(np.arange(WIN, dtype=np.float32), max_np)
        cst_host[:, 0:iota_cols] = iota[None, :]
    else:
        cst_host[:, 0:WIN] = np.arange(WIN, dtype=np.float32)[None, :]
    cst_host[:, iota_cols:] = np.eye(P, dtype=np.float32)
    gb_host = np.zeros((P, 2 * D), np.float32)
    gb_host[:, 0:D] = gamma[None, :]
    gb_host[:, D:2 * D] = beta[None, :]
    return wc_host, cst_host, gb_host


def kernel(feat, W0, W1, W2, a0, a1, a2, ln_gamma, ln_beta,
           src0, dst0, src1, dst1, src2, dst2):
    feat = np.asarray(feat, np.float32)
    prep = _host_prep(W0, W1, W2, a0, a1, a2,
                      [src0, src1, src2], [dst0, dst1, dst2])

    gamma = np.asarray(ln_gamma, np.float32).ravel()
    beta = np.asarray(ln_beta, np.float32).ravel()
    apply_affine = not (np.all(gamma == 1.0) and np.all(beta == 0.0))

    nc = _build_nc(prep["schedule"], prep["total_tiles"], prep["total_pairs"],
                   apply_affine)

    tab_bf16 = feat.astype(NP_BF16)
    wc_host, cst_host, gb_host = _host_tensors(prep, gamma, beta)

    in_maps = []
    for c in range(NC):
        in_maps.append({
            "tab_lo": tab_bf16[:LO_SPLIT],
            "tab_hi": tab_bf16[LO_SPLIT:],
            "idx": prep["idx16"][c],
            "dl": _bf16(prep["dlmat"][c]),
            "wg": _bf16(prep["wgmat"][c]),
            "wc": _bf16(wc_host),
            "cst": _bf16(cst_host),
            "gb": gb_host,
        })

    trace = os.environ.get("BENCH_TRACE", "0") == "1"
    kwargs = {}
    if trace:
        tmpdir = os.environ.get("BENCH_TRACE_DIR", "/tmp/kernel_trace")
        os.makedirs(tmpdir, exist_ok=True)
        kwargs = dict(trace=True, tmpdir=tmpdir)
    res = run_bass_kernel_spmd(nc, in_maps, core_ids=list(range(NC)), **kwargs)
    if trace and res.exec_time_ns:
        print(f"HW exec time: {res.exec_time_ns} ns")

    out = np.concatenate([np.asarray(res.results[c]["out"]) for c in range(NC)],
                         axis=0)
    return out[:N].astype(np.float32)


# revision 40
# speedup vs baseline: 1.1958x; 1.1958x over previous
"""Trainium2 Bass kernel for nn_AttentionHeteroRGCNLayer.

Math: softmax of a length-1 vector is 1.0, so the per-relation attention
weights are w = softmax([1,1,1]) = 1/3 each (computed generally anyway).
h = feat @ Wc with Wc = sum_r w_r W_r, and aggregation is linear, so the
layer is out = LN(relu(agg_feat @ Wc)) with per-edge weight
w_e = w_r / max(deg_r[dst_e], 1) folded into a one-hot scatter matrix:
    agg_feat[dst] = sum_e w_e * feat[src_e]

Distribution: dst-range sharding across 8 cores (6400 dst rows each, N padded
to 51200); the bf16 feat table is replicated to every core (no collectives).
Host buckets edges by dst into 256-dst "supers" (dense streams, split lo/hi
for dma_gather's int16 indices); the device gathers rows with dma_gather
(single-packet descriptor streams, calls balanced across the 4 SWDGE queues),
builds 64-wide one-hot scatter blocks per (tile, window) pair with broadcast
is_equal ops in a c-major layout (packed last dims keep the DVE in its fast
mode), and aggregates with bf16 matmuls into PSUM 64-row halves. Per 128-dst
block, Wc is applied via two PE transposes + two matmuls, then ReLU; LN
statistics accumulate via activation accumulators into [128, nblk] tiles and
are reduced in batches on the vector engine, with the final normalization a
single scalar-engine activation (per-partition scale/bias) per block.
"""
import os
import numpy as np
import ml_dtypes

import concourse.bacc as bacc
import concourse.bass as bass
import concourse.mybir as mybir
import concourse.tile as tile
from concourse.bass_utils import run_bass_kernel_spmd

BF16 = mybir.dt.bfloat16
F32 = mybir.dt.float32
NP_BF16 = np.dtype(ml_dtypes.bfloat16)

N = 50000
D = 256
P = 128
WIN = 64                 # one-hot window width
NC = 8
NPAD = 51200
SUPER_DST = 256                      # dsts per super
LO_SPLIT = 32768
MAX_TILES_PER_CALL = 17
LN_EPS = 1e-5
NQ = 4
GROUP_BLOCKS = 10        # LN stat batch size (128-dst blocks)

SINGLE_PACKET = os.environ.get("K_SINGLE_PACKET", "0") == "1"
CMAJOR = os.environ.get("K_CMAJOR", "1") == "1"
NOTRIM = os.environ.get("K_NOTRIM", "0") == "1"


def _rows_per_core():
    return NPAD // NC


def _supers_per_core():
    return _rows_per_core() // SUPER_DST


def _kwin():
    return SUPER_DST // WIN


def _bf16(x):
    return np.asarray(x, dtype=np.float32).astype(NP_BF16)


def _softmax(v):
    e = np.exp(v - v.max())
    return e / e.sum()


def _even_chunks(total, n):
    base, rem = divmod(total, n)
    out = []
    ofs = 0
    for i in range(n):
        c = base + (1 if i < rem else 0)
        out.append((ofs, c))
        ofs += c
    return out


def _plan_calls(tl, th):
    """Split a super's lo/hi tile streams into exactly NQ (or 2*NQ) gather
    calls. The tile framework assigns SWDGE sem lanes round-robin (8 lanes),
    so queue_num must follow the global issue cycle — callers map call i of
    super s to queue (i + off) % NQ with a fixed per-super call count.
    Returns [(kind, ofs, cnt), ...]."""
    for ncalls in (NQ, 2 * NQ):
        best = None
        for nlo in range(1, ncalls):
            nhi = ncalls - nlo
            if -(-tl // nlo) > MAX_TILES_PER_CALL or -(-th // nhi) > MAX_TILES_PER_CALL:
                continue
            if nlo > tl or nhi > th:
                continue
            chunks = ([("lo", o, c) for o, c in _even_chunks(tl, nlo)]
                      + [("hi", o, c) for o, c in _even_chunks(th, nhi)])
            sizes = [c for _, _, c in chunks]
            key = (max(sizes), sum(c * c for c in sizes))
            if best is None or key < best[0]:
                best = (key, chunks)
        if best is not None:
            return best[1]
    raise AssertionError(f"cannot plan calls for tl={tl} th={th}")


def _host_prep(W0, W1, W2, a0, a1, a2, srcs, dsts):
    supers_per_core = _supers_per_core()
    kwin = _kwin()
    w3 = _softmax(np.concatenate([_softmax(np.asarray(a, np.float64).ravel())
                                  for a in (a0, a1, a2)]))
    Wc = (w3[0] * np.asarray(W0, np.float32) + w3[1] * np.asarray(W1, np.float32)
          + w3[2] * np.asarray(W2, np.float32)).astype(np.float32)

    src_all, dst_all, wgt_all = [], [], []
    for r in range(3):
        s = np.asarray(srcs[r], np.int64)
        d = np.asarray(dsts[r], np.int64)
        deg = np.bincount(d, minlength=N)
        w_e = (w3[r] / np.maximum(deg, 1.0)[d]).astype(np.float32)
        src_all.append(s); dst_all.append(d); wgt_all.append(w_e)
    src_all = np.concatenate(src_all)
    dst_all = np.concatenate(dst_all)
    wgt_all = np.concatenate(wgt_all)

    order = np.argsort(dst_all, kind="stable")
    s_s, d_s, w_s = src_all[order], dst_all[order], wgt_all[order]

    # per (core, super) lo/hi streams: (src, dst_rel[0..256), wgt)
    gsup = d_s // SUPER_DST
    sup_counts = np.bincount(gsup, minlength=NC * supers_per_core)
    sup_start = np.zeros(NC * supers_per_core + 1, np.int64)
    np.cumsum(sup_counts, out=sup_start[1:])

    streams = {}     # (c, s, 'lo'/'hi') -> (src_idx, dst_rel, wgt)
    n_lo = np.zeros((NC, supers_per_core), np.int64)
    n_hi = np.zeros((NC, supers_per_core), np.int64)
    for g in range(NC * supers_per_core):
        c, s = g // supers_per_core, g % supers_per_core
        a, b = sup_start[g], sup_start[g + 1]
        sl_s, sl_d, sl_w = s_s[a:b], d_s[a:b], w_s[a:b]
        rel = sl_d - g * SUPER_DST
        m = sl_s < LO_SPLIT
        streams[(c, s, "lo")] = (sl_s[m], rel[m], sl_w[m])
        streams[(c, s, "hi")] = (sl_s[~m] - LO_SPLIT, rel[~m], sl_w[~m])
        n_lo[c, s] = int(m.sum())
        n_hi[c, s] = int((~m).sum())

    T_lo = np.maximum(1, -(-n_lo.max(axis=0) // P))
    T_hi = np.maximum(1, -(-n_hi.max(axis=0) // P))

    n_lo_max = n_lo.max(axis=0)
    n_hi_max = n_hi.max(axis=0)

    schedule = []
    total_tiles = 0
    total_pairs = 0
    for s in range(supers_per_core):
        tl, th = int(T_lo[s]), int(T_hi[s])
        ntile = tl + th
        nmax = {"lo": tl * P, "hi": th * P} if NOTRIM else \
            {"lo": int(n_lo_max[s]), "hi": int(n_hi_max[s])}
        calls = [(kind, ofs, cnt,
                  int(np.clip(nmax[kind] - ofs * P, 0, cnt * P)))
                 for kind, ofs, cnt in _plan_calls(tl, th)]
        # rotate emission so the big chunks cycle through the queues
        r = s % len(calls)
        calls = calls[r:] + calls[:r]
        # pair schedule: for each tile, cross-core k-window range
        tile_kmin = np.full(ntile, kwin, np.int64)
        tile_kmax = np.full(ntile, -1, np.int64)
        for c in range(NC):
            for kind, tbase, tcnt in (("lo", 0, tl), ("hi", tl, th)):
                rel = streams[(c, s, kind)][1]
                n = len(rel)
                if n == 0:
                    continue
                kk = rel // WIN
                for t in range(min(tcnt, -(-n // P))):
                    seg = kk[t * P:(t + 1) * P]
                    tile_kmin[tbase + t] = min(tile_kmin[tbase + t], int(seg.min()))
                    tile_kmax[tbase + t] = max(tile_kmax[tbase + t], int(seg.max()))
        pairs = []          # (tile, k)
        for t in range(ntile):
            if tile_kmax[t] < 0:
                continue
            for k in range(int(tile_kmin[t]), int(tile_kmax[t]) + 1):
                pairs.append((t, k))
        covered = {k for _, k in pairs}
        for k in range(kwin):
            if k not in covered:
                pairs.append((0, k))
        # group by k for clean PSUM accumulation chains, tiles in order
        by_k = {k: [] for k in range(kwin)}
        for t, k in pairs:
            by_k[k].append(t)
        pair_sched = []     # (k, tile, paircol)
        paircol = 0
        for k in range(kwin):
            for t in sorted(by_k[k]):
                pair_sched.append((k, t, total_pairs + paircol))
                paircol += 1
        npairs = paircol
        schedule.append(dict(
            tile_base=total_tiles, pair_base=total_pairs,
            tl=tl, th=th, ntile=ntile, calls=calls, nmax=nmax,
            pair_sched=pair_sched, npairs=npairs,
        ))
        total_tiles += ntile
        total_pairs += npairs

    # host metadata arrays
    idx16 = np.zeros((NC, P, total_tiles * 8), np.int16)
    dlmat = np.full((NC, P, total_pairs), -1000.0, np.float32)
    wgmat = np.zeros((NC, P, total_pairs), np.float32)

    for c in range(NC):
        for s in range(supers_per_core):
            sc = schedule[s]
            tl, th = sc["tl"], sc["th"]
            relcap = np.full((sc["ntile"], P), -1000.0, np.float32)
            wgtcap = np.zeros((sc["ntile"], P), np.float32)
            for kind, tbase, tcnt in (("lo", 0, tl), ("hi", tl, th)):
                es, rel, ew = streams[(c, s, kind)]
                n = len(es)
                cap = tcnt * P
                nmax = cap if NOTRIM else \
                    int((n_lo_max if kind == "lo" else n_hi_max)[s])
                # pad with row 0 up to the cross-core max valid count (the
                # per-call num_idxs_reg, shared by the SPMD program), -1 after
                # (ucode skips trailing negatives)
                sidx = np.zeros(cap, np.int64)
                sidx[:n] = es
                sidx[nmax:] = -1
                rl = np.full(cap, -1000.0, np.float32)
                rl[:n] = rel.astype(np.float32)
                wv = np.zeros(cap, np.float32)
                wv[:n] = ew
                relcap[tbase:tbase + tcnt] = rl.reshape(tcnt, P)
                wgtcap[tbase:tbase + tcnt] = wv.reshape(tcnt, P)
                iw = sidx.reshape(tcnt, 8, 16).transpose(0, 2, 1)   # [t,16,8]
                iw = np.tile(iw, (1, 8, 1))                          # [t,128,8]
                tb = sc["tile_base"] + tbase
                idx16[c, :, tb * 8:(tb + tcnt) * 8] = (
                    iw.transpose(1, 0, 2).reshape(P, tcnt * 8).astype(np.int16))
            for k, t, pc in sc["pair_sched"]:
                dlmat[c, :, pc] = relcap[t] - k * WIN
                wgmat[c, :, pc] = wgtcap[t]

    return dict(Wc=Wc, schedule=schedule, total_tiles=total_tiles,
                total_pairs=total_pairs, idx16=idx16, dlmat=dlmat, wgmat=wgmat)


def _build_nc(schedule, total_tiles, total_pairs, apply_affine):
    supers_per_core = _supers_per_core()
    rows_per_core = _rows_per_core()
    kwin = _kwin()
    nblk = 2 * supers_per_core
    nc = bacc.Bacc(None, target_bir_lowering=False, num_swdge_queues=NQ)
    tab_lo = nc.declare_dram_parameter("tab_lo", [LO_SPLIT, D], BF16, isOutput=False)
    tab_hi = nc.declare_dram_parameter("tab_hi", [N - LO_SPLIT, D], BF16, isOutput=False)
    idx_d = nc.declare_dram_parameter("idx", [P, total_tiles * 8], mybir.dt.int16, isOutput=False)
    dl_d = nc.declare_dram_parameter("dl", [P, total_pairs], BF16, isOutput=False)
    wg_d = nc.declare_dram_parameter("wg", [P, total_pairs], BF16, isOutput=False)
    wc_d = nc.declare_dram_parameter("wc", [P, 2 * D], BF16, isOutput=False)
    max_np = max(sc["npairs"] for sc in schedule)
    iota_cols = WIN * max_np if CMAJOR else WIN
    # cst: iota (c-major [WIN, max_np] or plain [WIN]) | identity
    cst_d = nc.declare_dram_parameter("cst", [P, iota_cols + P], BF16, isOutput=False)
    gb_d = nc.declare_dram_parameter("gb", [P, 2 * D], F32, isOutput=False)
    out_d = nc.declare_dram_parameter("out", [rows_per_core, D], BF16, isOutput=True)

    max_tl = max(sc["tl"] for sc in schedule)
    max_th = max(sc["th"] for sc in schedule)
    qrot = [0]

    with tile.TileContext(nc) as tc:
        with (
            tc.tile_pool(name="meta", bufs=1) as meta_pool,
            tc.tile_pool(name="xlo", bufs=4) as xlo_pool,
            tc.tile_pool(name="xhi", bufs=4) as xhi_pool,
            tc.tile_pool(name="bmat0", bufs=1) as b0_pool,
            tc.tile_pool(name="bmat", bufs=2) as b_pool,
            tc.tile_pool(name="ev", bufs=3) as ev_pool,
            tc.tile_pool(name="yo", bufs=3) as y_pool,
            tc.tile_pool(name="xst", bufs=2) as xst_pool,
            tc.tile_pool(name="stat", bufs=2) as stat_pool,
            tc.tile_pool(name="psA", bufs=3, space="PSUM") as psA,
            tc.tile_pool(name="psB", bufs=2, space="PSUM") as psB,
            tc.tile_pool(name="psC", bufs=2, space="PSUM") as psC,
        ):
            idx_sb = meta_pool.tile([P, total_tiles * 8], mybir.dt.int16)
            # chunked so the first supers' gathers don't wait on the full load
            idx_chunk = -(-total_tiles // 5) * 8
            for j in range(0, total_tiles * 8, idx_chunk):
                e = min(j + idx_chunk, total_tiles * 8)
                nc.sync.dma_start(out=idx_sb[:, j:e], in_=idx_d[:, j:e])
            mrow = meta_pool.tile([P, 2 * total_pairs + 2 * D + iota_cols + P], BF16)
            nc.sync.dma_start(out=mrow[:, :total_pairs], in_=dl_d[:])
            nc.sync.dma_start(out=mrow[:, total_pairs:2 * total_pairs], in_=wg_d[:])
            nc.sync.dma_start(out=mrow[:, 2 * total_pairs:2 * total_pairs + 2 * D], in_=wc_d[:])
            nc.sync.dma_start(out=mrow[:, 2 * total_pairs + 2 * D:], in_=cst_d[:])
            dl_sb = mrow[:, 0:total_pairs]
            wg_sb = mrow[:, total_pairs:2 * total_pairs]
            wc_sb = mrow[:, 2 * total_pairs:2 * total_pairs + 2 * D]
            iota_sb = mrow[:, 2 * total_pairs + 2 * D:2 * total_pairs + 2 * D + iota_cols]
            ident_sb = mrow[:, 2 * total_pairs + 2 * D + iota_cols:]
            gb_sb = meta_pool.tile([P, 2 * D], F32)
            nc.sync.dma_start(out=gb_sb[:], in_=gb_d[:])
            gamma_sb = gb_sb[:, 0:D]
            beta_sb = gb_sb[:, D:2 * D]



            eps_tile = meta_pool.tile([P, 1], F32)
            nc.vector.memset(eps_tile[:], LN_EPS)
            eps_col = eps_tile[:]

            # per-group LN staging/stat tiles (separate tiles, not slices of
            # one big tile: dependency tracking is tile-granular, so a shared
            # tile would make group reads wait on ALL writes issued so far,
            # head-of-line blocking the engine queues)
            GB = GROUP_BLOCKS
            group_tiles = {}

            def group_of(w):
                g = w // GB
                if g not in group_tiles:
                    xst_g = xst_pool.tile([P, GB * D], BF16, tag="xst")
                    s1_g = stat_pool.tile([P, GB], F32, tag="s1")
                    s2_g = stat_pool.tile([P, GB], F32, tag="s2")
                    st_g = stat_pool.tile([P, 5 * GB], F32, tag="st")
                    group_tiles[g] = dict(xst=xst_g, s1=s1_g, s2=s2_g, st=st_g)
                return group_tiles[g]

            def flush_group(g0, g1):
                n = g1 - g0
                if n <= 0:
                    return
                gt = group_tiles[g0 // GB]
                s1, s2 = gt["s1"][:, :n], gt["s2"][:, :n]
                st = gt["st"]
                mu = st[:, 0 * GB:0 * GB + n]
                mm = st[:, 1 * GB:1 * GB + n]
                ex2 = st[:, 2 * GB:2 * GB + n]
                var = ex2  # in-place: var = ex2 - mm
                sd = st[:, 3 * GB:3 * GB + n]
                rstd = st[:, 4 * GB:4 * GB + n]
                nmb = mm  # reuse mm slot for -mu*rstd (mm dead after var)
                # stat prologue on the scalar engine: DVE ops that read the
                # accumulator-written s1/s2 mid-stream stall the vector queue
                # for tens of us (contention quirk); scalar reads them fine
                nc.scalar.activation(out=mu, in_=s1,
                                     func=mybir.ActivationFunctionType.Copy,
                                     scale=1.0 / D)
                nc.scalar.activation(out=mm, in_=mu,
                                     func=mybir.ActivationFunctionType.Square)
                nc.scalar.activation(out=ex2, in_=s2,
                                     func=mybir.ActivationFunctionType.Copy,
                                     scale=1.0 / D)
                nc.vector.tensor_tensor(out=var, in0=ex2, in1=mm,
                                        op=mybir.AluOpType.subtract)
                nc.vector.tensor_scalar(out=var, in0=var,
                                        scalar1=0.0, scalar2=None,
                                        op0=mybir.AluOpType.max)
                nc.scalar.activation(out=sd, in_=var,
                                     func=mybir.ActivationFunctionType.Sqrt,
                                     bias=eps_col)
                nc.vector.reciprocal(out=rstd, in_=sd)
                nc.vector.scalar_tensor_tensor(out=nmb, in0=mu,
                                               scalar=-1.0, in1=rstd,
                                               op0=mybir.AluOpType.mult,
                                               op1=mybir.AluOpType.mult)
                for w in range(g0, g1):
                    j = w - g0
                    yout = y_pool.tile([P, D], BF16, tag="y")
                    nc.scalar.activation(out=yout[:],
                                         in_=gt["xst"][:, j * D:(j + 1) * D],
                                         func=mybir.ActivationFunctionType.Identity,
                                         scale=rstd[:, j:j + 1], bias=nmb[:, j:j + 1])
                    if apply_affine:
                        y2 = y_pool.tile([P, D], F32, tag="y2")
                        nc.vector.tensor_tensor(out=y2[:], in0=yout[:], in1=gamma_sb,
                                                op=mybir.AluOpType.mult)
                        y3 = y_pool.tile([P, D], BF16, tag="y3")
                        nc.vector.tensor_tensor(out=y3[:], in0=y2[:], in1=beta_sb,
                                                op=mybir.AluOpType.add)
                        yfin = y3
                    else:
                        yfin = yout
                    nc.sync.dma_start(out=out_d[w * P:(w + 1) * P, :], in_=yfin[:])

            done_blocks = 0
            for s in range(supers_per_core):
                sc = schedule[s]
                base = sc["tile_base"]
                pbase = sc["pair_base"]
                tl, th, npairs = sc["tl"], sc["th"], sc["npairs"]
                xlo = xlo_pool.tile([P, max_tl * D], BF16, tag="xlo")
                xhi = xhi_pool.tile([P, max_th * D], BF16, tag="xhi")
                # zero the pad tail the gather will skip (trailing -1 idxs):
                # the one-hot columns there are zero, but 0 * NaN from stale
                # slots would still poison the PSUM accumulate
                for kind, x_t, tcnt in (("lo", xlo, tl), ("hi", xhi, th)):
                    pt = sc["nmax"][kind] // P
                    if pt < tcnt:
                        nc.vector.memset(x_t[:, pt * D:tcnt * D], 0.0)
                for kind, ofs, cnt, reg in sc["calls"]:
                    x_t, tab, tofs = (xlo, tab_lo, base + ofs) if kind == "lo" \
                        else (xhi, tab_hi, base + tl + ofs)
                    ni = cnt * P
                    nc.gpsimd.dma_gather(
                        out_ap=x_t[:, ofs * D:(ofs + cnt) * D].rearrange(
                            "p (t e) -> p t e", e=D),
                        in_ap=tab[:],
                        idxs_ap=idx_sb[:, tofs * 8:(tofs + cnt) * 8],
                        num_idxs=ni, num_idxs_reg=reg, elem_size=D,
                        single_packet=SINGLE_PACKET,
                        queue_num=qrot[0] % NQ,
                    )
                    qrot[0] += 1

                if CMAJOR:
                    bmat0 = b0_pool.tile([P, max_np * WIN], BF16, tag="b0")
                    bmat = b_pool.tile([P, max_np * WIN], BF16, tag="b")
                    b0view = bmat0[:, :npairs * WIN].rearrange(
                        "p (c t) -> p c t", t=npairs)
                    bview = bmat[:, :npairs * WIN].rearrange(
                        "p (c t) -> p c t", t=npairs)
                    iview = iota_sb.rearrange(
                        "p (c t) -> p c t", t=max_np)[:, :, 0:npairs]
                    nc.vector.tensor_tensor(
                        out=b0view, in0=iview,
                        in1=dl_sb[:, pbase:pbase + npairs].unsqueeze(1).to_broadcast(
                            [P, WIN, npairs]),
                        op=mybir.AluOpType.is_equal,
                    )
                    nc.vector.tensor_tensor(
                        out=bview, in0=b0view,
                        in1=wg_sb[:, pbase:pbase + npairs].unsqueeze(1).to_broadcast(
                            [P, WIN, npairs]),
                        op=mybir.AluOpType.mult,
                    )

                    def lhsT_of(pc_rel):
                        return bmat[:, :npairs * WIN].rearrange(
                            "p (c t) -> p c t", t=npairs)[:, :, pc_rel:pc_rel + 1]
                else:
                    bmat = b_pool.tile([P, max_np * WIN], BF16, tag="b")
                    nc.vector.tensor_tensor(
                        out=bmat[:, :npairs * WIN].rearrange("p (t c) -> p t c", c=WIN),
                        in0=iota_sb.unsqueeze(1).to_broadcast([P, npairs, WIN]),
                        in1=dl_sb[:, pbase:pbase + npairs].unsqueeze(2).to_broadcast(
                            [P, npairs, WIN]),
                        op=mybir.AluOpType.is_equal,
                    )
                    nc.vector.tensor_tensor(
                        out=bmat[:, :npairs * WIN].rearrange("p (t c) -> p t c", c=WIN),
                        in0=bmat[:, :npairs * WIN].rearrange("p (t c) -> p t c", c=WIN),
                        in1=wg_sb[:, pbase:pbase + npairs].unsqueeze(2).to_broadcast(
                            [P, npairs, WIN]),
                        op=mybir.AluOpType.mult,
                    )

                    def lhsT_of(pc_rel):
                        return bmat[:, pc_rel * WIN:(pc_rel + 1) * WIN]

                # aggregation into PSUM 64-row halves; pair_sched grouped by k
                aggb = psA.tile([P, 2 * D], F32, tag="agg")
                aggs = [aggb[:, 0:D], aggb[:, D:2 * D]]
                ksched = {}
                for k, t, pc in sc["pair_sched"]:
                    ksched.setdefault(k, []).append((t, pc))
                for k in range(kwin):
                    plist = ksched[k]
                    half = k % 2
                    agg = aggs[k // 2]
                    for i, (t, pc) in enumerate(plist):
                        xsrc = xlo if t < tl else xhi
                        xoff = t if t < tl else t - tl
                        nc.tensor.matmul(
                            out=agg[half * WIN:(half + 1) * WIN, :],
                            lhsT=lhsT_of(pc - pbase),
                            rhs=xsrc[:, xoff * D:(xoff + 1) * D],
                            start=(i == 0), stop=(i == len(plist) - 1),
                        )

                for b in range(2):
                    w = 2 * s + b          # 128-dst evac block
                    agg = aggs[b]
                    aggS = ev_pool.tile([P, D], BF16, tag="aggS")
                    nc.scalar.activation(out=aggS[:], in_=agg[:],
                                         func=mybir.ActivationFunctionType.Copy)
                    trp = psB.tile([P, D], BF16, tag="trp")
                    nc.tensor.transpose(out=trp[:, 0:P], in_=aggS[:, 0:P], identity=ident_sb)
                    nc.tensor.transpose(out=trp[:, P:D], in_=aggS[:, P:D], identity=ident_sb)
                    aggT = ev_pool.tile([P, D], BF16, tag="aggT")
                    nc.scalar.activation(out=aggT[:], in_=trp[:],
                                         func=mybir.ActivationFunctionType.Copy)
                    out2 = psC.tile([P, D], F32, tag="out2")
                    nc.tensor.matmul(out=out2[:], lhsT=aggT[:, 0:P],
                                     rhs=wc_sb[:, 0:D], start=True, stop=False)
                    nc.tensor.matmul(out=out2[:], lhsT=aggT[:, P:D],
                                     rhs=wc_sb[:, D:2 * D], start=False, stop=True)
                    # ReLU into bf16 staging, accumulate sum into s1
                    gt = group_of(w)
                    j = w % GB
                    nc.scalar.activation(out=gt["xst"][:, j * D:(j + 1) * D],
                                         in_=out2[:],
                                         func=mybir.ActivationFunctionType.Relu,
                                         accum_out=gt["s1"][:, j:j + 1])
                    xsq = ev_pool.tile([P, D], BF16, tag="xsq")
                    nc.scalar.activation(out=xsq[:],
                                         in_=gt["xst"][:, j * D:(j + 1) * D],
                                         func=mybir.ActivationFunctionType.Square,
                                         accum_out=gt["s2"][:, j:j + 1])

                # flush LN groups with a 2-super delay so the stat reads never
                # head-of-line block the vector queue behind pending
                # accumulator writes (stalls the whole gather pipeline)
                blocks_ready = 2 * (s - 1)
                while blocks_ready - done_blocks >= GROUP_BLOCKS:
                    flush_group(done_blocks, done_blocks + GROUP_BLOCKS)
                    done_blocks += GROUP_BLOCKS
            while done_blocks < nblk:
                g1 = min(done_blocks + GROUP_BLOCKS, nblk)
                flush_group(done_blocks, g1)
                done_blocks = g1
    nc.compile()
    return nc


def _host_tensors(prep, gamma, beta):
    max_np = max(sc["npairs"] for sc in prep["schedule"])
    wc_host = np.zeros((P, 2 * D), np.float32)
    wc_host[:, 0:D] = prep["Wc"][0:P, :]
    wc_host[:, D:2 * D] = prep["Wc"][P:D, :]
    iota_cols = WIN * max_np if CMAJOR else WIN
    cst_host = np.zeros((P, iota_cols + P), np.float32)
    if CMAJOR:
        iota = np.repeat(np.arange(WIN, dtype=np.float32), max_np)
        cst_host[:, 0:iota_cols] = iota[None, :]
    else:
        cst_host[:, 0:WIN] = np.arange(WIN, dtype=np.float32)[None, :]
    cst_host[:, iota_cols:] = np.eye(P, dtype=np.float32)
    gb_host = np.zeros((P, 2 * D), np.float32)
    gb_host[:, 0:D] = gamma[None, :]
    gb_host[:, D:2 * D] = beta[None, :]
    return wc_host, cst_host, gb_host


def kernel(feat, W0, W1, W2, a0, a1, a2, ln_gamma, ln_beta,
           src0, dst0, src1, dst1, src2, dst2):
    feat = np.asarray(feat, np.float32)
    prep = _host_prep(W0, W1, W2, a0, a1, a2,
                      [src0, src1, src2], [dst0, dst1, dst2])

    gamma = np.asarray(ln_gamma, np.float32).ravel()
    beta = np.asarray(ln_beta, np.float32).ravel()
    apply_affine = not (np.all(gamma == 1.0) and np.all(beta == 0.0))

    nc = _build_nc(prep["schedule"], prep["total_tiles"], prep["total_pairs"],
                   apply_affine)

    tab_bf16 = feat.astype(NP_BF16)
    wc_host, cst_host, gb_host = _host_tensors(prep, gamma, beta)

    in_maps = []
    for c in range(NC):
        in_maps.append({
            "tab_lo": tab_bf16[:LO_SPLIT],
            "tab_hi": tab_bf16[LO_SPLIT:],
            "idx": prep["idx16"][c],
            "dl": _bf16(prep["dlmat"][c]),
            "wg": _bf16(prep["wgmat"][c]),
            "wc": _bf16(wc_host),
            "cst": _bf16(cst_host),
            "gb": gb_host,
        })

    trace = os.environ.get("BENCH_TRACE", "0") == "1"
    kwargs = {}
    if trace:
        tmpdir = os.environ.get("BENCH_TRACE_DIR", "/tmp/kernel_trace")
        os.makedirs(tmpdir, exist_ok=True)
        kwargs = dict(trace=True, tmpdir=tmpdir)
    res = run_bass_kernel_spmd(nc, in_maps, core_ids=list(range(NC)), **kwargs)
    if trace and res.exec_time_ns:
        print(f"HW exec time: {res.exec_time_ns} ns")

    out = np.concatenate([np.asarray(res.results[c]["out"]) for c in range(NC)],
                         axis=0)
    return out[:N].astype(np.float32)


# revision 41
# speedup vs baseline: 1.2816x; 1.0717x over previous
"""Trainium2 Bass kernel for nn_AttentionHeteroRGCNLayer.

Math: softmax of a length-1 vector is 1.0, so the per-relation attention
weights are w = softmax([1,1,1]) = 1/3 each (computed generally anyway).
h = feat @ Wc with Wc = sum_r w_r W_r, and aggregation is linear, so the
layer is out = LN(relu(agg_feat @ Wc)) with per-edge weight
w_e = w_r / max(deg_r[dst_e], 1) folded into a one-hot scatter matrix:
    agg_feat[dst] = sum_e w_e * feat[src_e]

Distribution: dst-range sharding across 8 cores (6400 dst rows each, N padded
to 51200); the bf16 feat table is replicated to every core (no collectives).
Host buckets edges by dst into 256-dst "supers" (dense streams, split lo/hi
for dma_gather's int16 indices); the device gathers rows with dma_gather
(single-packet descriptor streams, calls balanced across the 4 SWDGE queues),
builds 64-wide one-hot scatter blocks per (tile, window) pair with broadcast
is_equal ops in a c-major layout (packed last dims keep the DVE in its fast
mode), and aggregates with bf16 matmuls into PSUM 64-row halves. Per 128-dst
block, Wc is applied via two PE transposes + two matmuls, then ReLU; LN
statistics accumulate via activation accumulators into [128, nblk] tiles and
are reduced in batches on the vector engine, with the final normalization a
single scalar-engine activation (per-partition scale/bias) per block.
"""
import os
import numpy as np
import ml_dtypes

import concourse.bacc as bacc
import concourse.bass as bass
import concourse.mybir as mybir
import concourse.tile as tile
from concourse.bass_utils import run_bass_kernel_spmd

BF16 = mybir.dt.bfloat16
F32 = mybir.dt.float32
NP_BF16 = np.dtype(ml_dtypes.bfloat16)

N = 50000
D = 256
P = 128
WIN = 64                 # one-hot window width
NC = 8
NPAD = 51200
SUPER_DST = 256                      # dsts per super
LO_SPLIT = 32768
MAX_TILES_PER_CALL = 17
LN_EPS = 1e-5
NQ = 4
GROUP_BLOCKS = 10        # LN stat batch size (128-dst blocks)

SINGLE_PACKET = os.environ.get("K_SINGLE_PACKET", "0") == "1"
CMAJOR = os.environ.get("K_CMAJOR", "1") == "1"
NOTRIM = os.environ.get("K_NOTRIM", "0") == "1"


def _rows_per_core():
    return NPAD // NC


def _supers_per_core():
    return _rows_per_core() // SUPER_DST


def _kwin():
    return SUPER_DST // WIN


def _bf16(x):
    return np.asarray(x, dtype=np.float32).astype(NP_BF16)


def _softmax(v):
    e = np.exp(v - v.max())
    return e / e.sum()


def _even_chunks(total, n):
    base, rem = divmod(total, n)
    out = []
    ofs = 0
    for i in range(n):
        c = base + (1 if i < rem else 0)
        out.append((ofs, c))
        ofs += c
    return out


def _plan_calls(tl, th):
    """Split a super's lo/hi tile streams into exactly NQ (or 2*NQ) gather
    calls. The tile framework assigns SWDGE sem lanes round-robin (8 lanes),
    so queue_num must follow the global issue cycle — callers map call i of
    super s to queue (i + off) % NQ with a fixed per-super call count.
    Returns [(kind, ofs, cnt), ...]."""
    for ncalls in (NQ, 2 * NQ):
        best = None
        for nlo in range(1, ncalls):
            nhi = ncalls - nlo
            if -(-tl // nlo) > MAX_TILES_PER_CALL or -(-th // nhi) > MAX_TILES_PER_CALL:
                continue
            if nlo > tl or nhi > th:
                continue
            chunks = ([("lo", o, c) for o, c in _even_chunks(tl, nlo)]
                      + [("hi", o, c) for o, c in _even_chunks(th, nhi)])
            sizes = [c for _, _, c in chunks]
            key = (max(sizes), sum(c * c for c in sizes))
            if best is None or key < best[0]:
                best = (key, chunks)
        if best is not None:
            return best[1]
    raise AssertionError(f"cannot plan calls for tl={tl} th={th}")


def _host_prep(W0, W1, W2, a0, a1, a2, srcs, dsts):
    supers_per_core = _supers_per_core()
    kwin = _kwin()
    w3 = _softmax(np.concatenate([_softmax(np.asarray(a, np.float64).ravel())
                                  for a in (a0, a1, a2)]))
    Wc = (w3[0] * np.asarray(W0, np.float32) + w3[1] * np.asarray(W1, np.float32)
          + w3[2] * np.asarray(W2, np.float32)).astype(np.float32)

    src_all, dst_all, wgt_all = [], [], []
    for r in range(3):
        s = np.asarray(srcs[r], np.int64)
        d = np.asarray(dsts[r], np.int64)
        deg = np.bincount(d, minlength=N)
        w_e = (w3[r] / np.maximum(deg, 1.0)[d]).astype(np.float32)
        src_all.append(s); dst_all.append(d); wgt_all.append(w_e)
    src_all = np.concatenate(src_all)
    dst_all = np.concatenate(dst_all)
    wgt_all = np.concatenate(wgt_all)

    order = np.argsort(dst_all, kind="stable")
    s_s, d_s, w_s = src_all[order], dst_all[order], wgt_all[order]

    # per (core, super) lo/hi streams: (src, dst_rel[0..256), wgt)
    gsup = d_s // SUPER_DST
    sup_counts = np.bincount(gsup, minlength=NC * supers_per_core)
    sup_start = np.zeros(NC * supers_per_core + 1, np.int64)
    np.cumsum(sup_counts, out=sup_start[1:])

    streams = {}     # (c, s, 'lo'/'hi') -> (src_idx, dst_rel, wgt)
    n_lo = np.zeros((NC, supers_per_core), np.int64)
    n_hi = np.zeros((NC, supers_per_core), np.int64)
    for g in range(NC * supers_per_core):
        c, s = g // supers_per_core, g % supers_per_core
        a, b = sup_start[g], sup_start[g + 1]
        sl_s, sl_d, sl_w = s_s[a:b], d_s[a:b], w_s[a:b]
        rel = sl_d - g * SUPER_DST
        m = sl_s < LO_SPLIT
        streams[(c, s, "lo")] = (sl_s[m], rel[m], sl_w[m])
        streams[(c, s, "hi")] = (sl_s[~m] - LO_SPLIT, rel[~m], sl_w[~m])
        n_lo[c, s] = int(m.sum())
        n_hi[c, s] = int((~m).sum())

    T_lo = np.maximum(1, -(-n_lo.max(axis=0) // P))
    T_hi = np.maximum(1, -(-n_hi.max(axis=0) // P))

    n_lo_max = n_lo.max(axis=0)
    n_hi_max = n_hi.max(axis=0)

    schedule = []
    total_tiles = 0
    total_pairs = 0
    for s in range(supers_per_core):
        tl, th = int(T_lo[s]), int(T_hi[s])
        ntile = tl + th
        nmax = {"lo": tl * P, "hi": th * P} if NOTRIM else \
            {"lo": int(n_lo_max[s]), "hi": int(n_hi_max[s])}
        calls = [(kind, ofs, cnt,
                  int(np.clip(nmax[kind] - ofs * P, 0, cnt * P)))
                 for kind, ofs, cnt in _plan_calls(tl, th)]
        # rotate emission so the big chunks cycle through the queues
        r = s % len(calls)
        calls = calls[r:] + calls[:r]
        # pair schedule: for each tile, cross-core k-window range
        tile_kmin = np.full(ntile, kwin, np.int64)
        tile_kmax = np.full(ntile, -1, np.int64)
        for c in range(NC):
            for kind, tbase, tcnt in (("lo", 0, tl), ("hi", tl, th)):
                rel = streams[(c, s, kind)][1]
                n = len(rel)
                if n == 0:
                    continue
                kk = rel // WIN
                for t in range(min(tcnt, -(-n // P))):
                    seg = kk[t * P:(t + 1) * P]
                    tile_kmin[tbase + t] = min(tile_kmin[tbase + t], int(seg.min()))
                    tile_kmax[tbase + t] = max(tile_kmax[tbase + t], int(seg.max()))
        pairs = []          # (tile, k)
        for t in range(ntile):
            if tile_kmax[t] < 0:
                continue
            for k in range(int(tile_kmin[t]), int(tile_kmax[t]) + 1):
                pairs.append((t, k))
        covered = {k for _, k in pairs}
        for k in range(kwin):
            if k not in covered:
                pairs.append((0, k))
        # group by k for clean PSUM accumulation chains, tiles in order
        by_k = {k: [] for k in range(kwin)}
        for t, k in pairs:
            by_k[k].append(t)
        pair_sched = []     # (k, tile, paircol)
        paircol = 0
        for k in range(kwin):
            for t in sorted(by_k[k]):
                pair_sched.append((k, t, total_pairs + paircol))
                paircol += 1
        npairs = paircol
        schedule.append(dict(
            tile_base=total_tiles, pair_base=total_pairs,
            tl=tl, th=th, ntile=ntile, calls=calls, nmax=nmax,
            pair_sched=pair_sched, npairs=npairs,
        ))
        total_tiles += ntile
        total_pairs += npairs

    # host metadata arrays
    idx16 = np.zeros((NC, P, total_tiles * 8), np.int16)
    dlmat = np.full((NC, P, total_pairs), -1000.0, np.float32)
    wgmat = np.zeros((NC, P, total_pairs), np.float32)

    for c in range(NC):
        for s in range(supers_per_core):
            sc = schedule[s]
            tl, th = sc["tl"], sc["th"]
            relcap = np.full((sc["ntile"], P), -1000.0, np.float32)
            wgtcap = np.zeros((sc["ntile"], P), np.float32)
            for kind, tbase, tcnt in (("lo", 0, tl), ("hi", tl, th)):
                es, rel, ew = streams[(c, s, kind)]
                n = len(es)
                cap = tcnt * P
                nmax = cap if NOTRIM else \
                    int((n_lo_max if kind == "lo" else n_hi_max)[s])
                # pad with row 0 up to the cross-core max valid count (the
                # per-call num_idxs_reg, shared by the SPMD program), -1 after
                # (ucode skips trailing negatives)
                sidx = np.zeros(cap, np.int64)
                sidx[:n] = es
                sidx[nmax:] = -1
                rl = np.full(cap, -1000.0, np.float32)
                rl[:n] = rel.astype(np.float32)
                wv = np.zeros(cap, np.float32)
                wv[:n] = ew
                relcap[tbase:tbase + tcnt] = rl.reshape(tcnt, P)
                wgtcap[tbase:tbase + tcnt] = wv.reshape(tcnt, P)
                iw = sidx.reshape(tcnt, 8, 16).transpose(0, 2, 1)   # [t,16,8]
                iw = np.tile(iw, (1, 8, 1))                          # [t,128,8]
                tb = sc["tile_base"] + tbase
                idx16[c, :, tb * 8:(tb + tcnt) * 8] = (
                    iw.transpose(1, 0, 2).reshape(P, tcnt * 8).astype(np.int16))
            for k, t, pc in sc["pair_sched"]:
                dlmat[c, :, pc] = relcap[t] - k * WIN
                wgmat[c, :, pc] = wgtcap[t]

    return dict(Wc=Wc, schedule=schedule, total_tiles=total_tiles,
                total_pairs=total_pairs, idx16=idx16, dlmat=dlmat, wgmat=wgmat)


def _build_nc(schedule, total_tiles, total_pairs, apply_affine):
    supers_per_core = _supers_per_core()
    rows_per_core = _rows_per_core()
    kwin = _kwin()
    nblk = 2 * supers_per_core
    nc = bacc.Bacc(None, target_bir_lowering=False, num_swdge_queues=NQ)
    tab_lo = nc.declare_dram_parameter("tab_lo", [LO_SPLIT, D], BF16, isOutput=False)
    tab_hi = nc.declare_dram_parameter("tab_hi", [N - LO_SPLIT, D], BF16, isOutput=False)
    idx_d = nc.declare_dram_parameter("idx", [P, total_tiles * 8], mybir.dt.int16, isOutput=False)
    dl_d = nc.declare_dram_parameter("dl", [P, total_pairs], BF16, isOutput=False)
    wg_d = nc.declare_dram_parameter("wg", [P, total_pairs], BF16, isOutput=False)
    wc_d = nc.declare_dram_parameter("wc", [P, 2 * D], BF16, isOutput=False)
    max_np = max(sc["npairs"] for sc in schedule)
    iota_cols = WIN * max_np if CMAJOR else WIN
    # cst: iota (c-major [WIN, max_np] or plain [WIN]) | identity
    cst_d = nc.declare_dram_parameter("cst", [P, iota_cols + P], BF16, isOutput=False)
    gb_d = nc.declare_dram_parameter("gb", [P, 2 * D], F32, isOutput=False)
    out_d = nc.declare_dram_parameter("out", [rows_per_core, D], BF16, isOutput=True)

    max_tl = max(sc["tl"] for sc in schedule)
    max_th = max(sc["th"] for sc in schedule)
    qrot = [0]

    with tile.TileContext(nc) as tc:
        with (
            tc.tile_pool(name="meta", bufs=1) as meta_pool,
            tc.tile_pool(name="xlo", bufs=4) as xlo_pool,
            tc.tile_pool(name="xhi", bufs=4) as xhi_pool,
            tc.tile_pool(name="bmat0", bufs=1) as b0_pool,
            tc.tile_pool(name="bmat", bufs=2) as b_pool,
            tc.tile_pool(name="ev", bufs=3) as ev_pool,
            tc.tile_pool(name="yo", bufs=3) as y_pool,
            tc.tile_pool(name="xst", bufs=2) as xst_pool,
            tc.tile_pool(name="stat", bufs=2) as stat_pool,
            tc.tile_pool(name="psA", bufs=3, space="PSUM") as psA,
            tc.tile_pool(name="psB", bufs=2, space="PSUM") as psB,
            tc.tile_pool(name="psC", bufs=2, space="PSUM") as psC,
        ):
            idx_sb = meta_pool.tile([P, total_tiles * 8], mybir.dt.int16)
            # chunked so the first supers' gathers don't wait on the full load
            idx_chunk = -(-total_tiles // 5) * 8
            for j in range(0, total_tiles * 8, idx_chunk):
                e = min(j + idx_chunk, total_tiles * 8)
                nc.sync.dma_start(out=idx_sb[:, j:e], in_=idx_d[:, j:e])
            mrow = meta_pool.tile([P, 2 * total_pairs + 2 * D + iota_cols + P], BF16)
            nc.sync.dma_start(out=mrow[:, :total_pairs], in_=dl_d[:])
            nc.sync.dma_start(out=mrow[:, total_pairs:2 * total_pairs], in_=wg_d[:])
            nc.sync.dma_start(out=mrow[:, 2 * total_pairs:2 * total_pairs + 2 * D], in_=wc_d[:])
            nc.sync.dma_start(out=mrow[:, 2 * total_pairs + 2 * D:], in_=cst_d[:])
            dl_sb = mrow[:, 0:total_pairs]
            wg_sb = mrow[:, total_pairs:2 * total_pairs]
            wc_sb = mrow[:, 2 * total_pairs:2 * total_pairs + 2 * D]
            iota_sb = mrow[:, 2 * total_pairs + 2 * D:2 * total_pairs + 2 * D + iota_cols]
            ident_sb = mrow[:, 2 * total_pairs + 2 * D + iota_cols:]
            gb_sb = meta_pool.tile([P, 2 * D], F32)
            nc.sync.dma_start(out=gb_sb[:], in_=gb_d[:])
            gamma_sb = gb_sb[:, 0:D]
            beta_sb = gb_sb[:, D:2 * D]



            eps_tile = meta_pool.tile([P, 1], F32)
            nc.vector.memset(eps_tile[:], LN_EPS)
            eps_col = eps_tile[:]

            # per-group LN staging/stat tiles (separate tiles, not slices of
            # one big tile: dependency tracking is tile-granular, so a shared
            # tile would make group reads wait on ALL writes issued so far,
            # head-of-line blocking the engine queues)
            GB = GROUP_BLOCKS
            group_tiles = {}

            def group_of(w):
                g = w // GB
                if g not in group_tiles:
                    xst_g = xst_pool.tile([P, GB * D], BF16, tag="xst")
                    s1_g = stat_pool.tile([P, GB], F32, tag="s1")
                    s2_g = stat_pool.tile([P, GB], F32, tag="s2")
                    st_g = stat_pool.tile([P, 5 * GB], F32, tag="st")
                    group_tiles[g] = dict(xst=xst_g, s1=s1_g, s2=s2_g, st=st_g)
                return group_tiles[g]

            def flush_group(g0, g1):
                n = g1 - g0
                if n <= 0:
                    return
                gt = group_tiles[g0 // GB]
                s1, s2 = gt["s1"][:, :n], gt["s2"][:, :n]
                st = gt["st"]
                mu = st[:, 0 * GB:0 * GB + n]
                mm = st[:, 1 * GB:1 * GB + n]
                ex2 = st[:, 2 * GB:2 * GB + n]
                var = ex2  # in-place: var = ex2 - mm
                sd = st[:, 3 * GB:3 * GB + n]
                rstd = st[:, 4 * GB:4 * GB + n]
                nmb = mm  # reuse mm slot for -mu*rstd (mm dead after var)
                # stat prologue on the scalar engine: DVE ops that read the
                # accumulator-written s1/s2 mid-stream stall the vector queue
                # for tens of us (contention quirk); scalar reads them fine
                nc.scalar.activation(out=mu, in_=s1,
                                     func=mybir.ActivationFunctionType.Copy,
                                     scale=1.0 / D)
                nc.scalar.activation(out=mm, in_=mu,
                                     func=mybir.ActivationFunctionType.Square)
                nc.scalar.activation(out=ex2, in_=s2,
                                     func=mybir.ActivationFunctionType.Copy,
                                     scale=1.0 / D)
                nc.vector.tensor_tensor(out=var, in0=ex2, in1=mm,
                                        op=mybir.AluOpType.subtract)
                # clamp via scalar Relu: a DVE tensor_scalar here (scalar-
                # from-pointer) stalls the vector queue for tens of us when
                # the SWDGE rings are busy
                nc.scalar.activation(out=sd, in_=var,
                                     func=mybir.ActivationFunctionType.Relu)
                nc.scalar.activation(out=sd, in_=sd,
                                     func=mybir.ActivationFunctionType.Sqrt,
                                     bias=eps_col)
                nc.vector.reciprocal(out=rstd, in_=sd)
                nc.vector.scalar_tensor_tensor(out=nmb, in0=mu,
                                               scalar=-1.0, in1=rstd,
                                               op0=mybir.AluOpType.mult,
                                               op1=mybir.AluOpType.mult)
                for w in range(g0, g1):
                    j = w - g0
                    yout = y_pool.tile([P, D], BF16, tag="y")
                    nc.scalar.activation(out=yout[:],
                                         in_=gt["xst"][:, j * D:(j + 1) * D],
                                         func=mybir.ActivationFunctionType.Identity,
                                         scale=rstd[:, j:j + 1], bias=nmb[:, j:j + 1])
                    if apply_affine:
                        y2 = y_pool.tile([P, D], F32, tag="y2")
                        nc.vector.tensor_tensor(out=y2[:], in0=yout[:], in1=gamma_sb,
                                                op=mybir.AluOpType.mult)
                        y3 = y_pool.tile([P, D], BF16, tag="y3")
                        nc.vector.tensor_tensor(out=y3[:], in0=y2[:], in1=beta_sb,
                                                op=mybir.AluOpType.add)
                        yfin = y3
                    else:
                        yfin = yout
                    nc.sync.dma_start(out=out_d[w * P:(w + 1) * P, :], in_=yfin[:])

            done_blocks = 0
            for s in range(supers_per_core):
                sc = schedule[s]
                base = sc["tile_base"]
                pbase = sc["pair_base"]
                tl, th, npairs = sc["tl"], sc["th"], sc["npairs"]
                xlo = xlo_pool.tile([P, max_tl * D], BF16, tag="xlo")
                xhi = xhi_pool.tile([P, max_th * D], BF16, tag="xhi")
                # zero the pad tail the gather will skip (trailing -1 idxs):
                # the one-hot columns there are zero, but 0 * NaN from stale
                # slots would still poison the PSUM accumulate
                for kind, x_t, tcnt in (("lo", xlo, tl), ("hi", xhi, th)):
                    pt = sc["nmax"][kind] // P
                    if pt < tcnt:
                        nc.vector.memset(x_t[:, pt * D:tcnt * D], 0.0)
                for kind, ofs, cnt, reg in sc["calls"]:
                    x_t, tab, tofs = (xlo, tab_lo, base + ofs) if kind == "lo" \
                        else (xhi, tab_hi, base + tl + ofs)
                    ni = cnt * P
                    nc.gpsimd.dma_gather(
                        out_ap=x_t[:, ofs * D:(ofs + cnt) * D].rearrange(
                            "p (t e) -> p t e", e=D),
                        in_ap=tab[:],
                        idxs_ap=idx_sb[:, tofs * 8:(tofs + cnt) * 8],
                        num_idxs=ni, num_idxs_reg=reg, elem_size=D,
                        single_packet=SINGLE_PACKET,
                        queue_num=qrot[0] % NQ,
                    )
                    qrot[0] += 1

                if CMAJOR:
                    bmat0 = b0_pool.tile([P, max_np * WIN], BF16, tag="b0")
                    bmat = b_pool.tile([P, max_np * WIN], BF16, tag="b")
                    b0view = bmat0[:, :npairs * WIN].rearrange(
                        "p (c t) -> p c t", t=npairs)
                    bview = bmat[:, :npairs * WIN].rearrange(
                        "p (c t) -> p c t", t=npairs)
                    iview = iota_sb.rearrange(
                        "p (c t) -> p c t", t=max_np)[:, :, 0:npairs]
                    nc.vector.tensor_tensor(
                        out=b0view, in0=iview,
                        in1=dl_sb[:, pbase:pbase + npairs].unsqueeze(1).to_broadcast(
                            [P, WIN, npairs]),
                        op=mybir.AluOpType.is_equal,
                    )
                    nc.vector.tensor_tensor(
                        out=bview, in0=b0view,
                        in1=wg_sb[:, pbase:pbase + npairs].unsqueeze(1).to_broadcast(
                            [P, WIN, npairs]),
                        op=mybir.AluOpType.mult,
                    )

                    def lhsT_of(pc_rel):
                        return bmat[:, :npairs * WIN].rearrange(
                            "p (c t) -> p c t", t=npairs)[:, :, pc_rel:pc_rel + 1]
                else:
                    bmat = b_pool.tile([P, max_np * WIN], BF16, tag="b")
                    nc.vector.tensor_tensor(
                        out=bmat[:, :npairs * WIN].rearrange("p (t c) -> p t c", c=WIN),
                        in0=iota_sb.unsqueeze(1).to_broadcast([P, npairs, WIN]),
                        in1=dl_sb[:, pbase:pbase + npairs].unsqueeze(2).to_broadcast(
                            [P, npairs, WIN]),
                        op=mybir.AluOpType.is_equal,
                    )
                    nc.vector.tensor_tensor(
                        out=bmat[:, :npairs * WIN].rearrange("p (t c) -> p t c", c=WIN),
                        in0=bmat[:, :npairs * WIN].rearrange("p (t c) -> p t c", c=WIN),
                        in1=wg_sb[:, pbase:pbase + npairs].unsqueeze(2).to_broadcast(
                            [P, npairs, WIN]),
                        op=mybir.AluOpType.mult,
                    )

                    def lhsT_of(pc_rel):
                        return bmat[:, pc_rel * WIN:(pc_rel + 1) * WIN]

                # aggregation into PSUM 64-row halves; pair_sched grouped by k
                aggb = psA.tile([P, 2 * D], F32, tag="agg")
                aggs = [aggb[:, 0:D], aggb[:, D:2 * D]]
                ksched = {}
                for k, t, pc in sc["pair_sched"]:
                    ksched.setdefault(k, []).append((t, pc))
                for k in range(kwin):
                    plist = ksched[k]
                    half = k % 2
                    agg = aggs[k // 2]
                    for i, (t, pc) in enumerate(plist):
                        xsrc = xlo if t < tl else xhi
                        xoff = t if t < tl else t - tl
                        nc.tensor.matmul(
                            out=agg[half * WIN:(half + 1) * WIN, :],
                            lhsT=lhsT_of(pc - pbase),
                            rhs=xsrc[:, xoff * D:(xoff + 1) * D],
                            start=(i == 0), stop=(i == len(plist) - 1),
                        )

                for b in range(2):
                    w = 2 * s + b          # 128-dst evac block
                    agg = aggs[b]
                    aggS = ev_pool.tile([P, D], BF16, tag="aggS")
                    nc.scalar.activation(out=aggS[:], in_=agg[:],
                                         func=mybir.ActivationFunctionType.Copy)
                    trp = psB.tile([P, D], BF16, tag="trp")
                    nc.tensor.transpose(out=trp[:, 0:P], in_=aggS[:, 0:P], identity=ident_sb)
                    nc.tensor.transpose(out=trp[:, P:D], in_=aggS[:, P:D], identity=ident_sb)
                    aggT = ev_pool.tile([P, D], BF16, tag="aggT")
                    nc.scalar.activation(out=aggT[:], in_=trp[:],
                                         func=mybir.ActivationFunctionType.Copy)
                    out2 = psC.tile([P, D], F32, tag="out2")
                    nc.tensor.matmul(out=out2[:], lhsT=aggT[:, 0:P],
                                     rhs=wc_sb[:, 0:D], start=True, stop=False)
                    nc.tensor.matmul(out=out2[:], lhsT=aggT[:, P:D],
                                     rhs=wc_sb[:, D:2 * D], start=False, stop=True)
                    # ReLU into bf16 staging, accumulate sum into s1
                    gt = group_of(w)
                    j = w % GB
                    nc.scalar.activation(out=gt["xst"][:, j * D:(j + 1) * D],
                                         in_=out2[:],
                                         func=mybir.ActivationFunctionType.Relu,
                                         accum_out=gt["s1"][:, j:j + 1])
                    xsq = ev_pool.tile([P, D], BF16, tag="xsq")
                    nc.scalar.activation(out=xsq[:],
                                         in_=gt["xst"][:, j * D:(j + 1) * D],
                                         func=mybir.ActivationFunctionType.Square,
                                         accum_out=gt["s2"][:, j:j + 1])

                # flush LN groups with a 2-super delay so the stat reads never
                # head-of-line block the vector queue behind pending
                # accumulator writes (stalls the whole gather pipeline)
                blocks_ready = 2 * (s - 1)
                while blocks_ready - done_blocks >= GROUP_BLOCKS:
                    flush_group(done_blocks, done_blocks + GROUP_BLOCKS)
                    done_blocks += GROUP_BLOCKS
            while done_blocks < nblk:
                g1 = min(done_blocks + GROUP_BLOCKS, nblk)
                flush_group(done_blocks, g1)
                done_blocks = g1
    nc.compile()
    return nc


def _host_tensors(prep, gamma, beta):
    max_np = max(sc["npairs"] for sc in prep["schedule"])
    wc_host = np.zeros((P, 2 * D), np.float32)
    wc_host[:, 0:D] = prep["Wc"][0:P, :]
    wc_host[:, D:2 * D] = prep["Wc"][P:D, :]
    iota_cols = WIN * max_np if CMAJOR else WIN
    cst_host = np.zeros((P, iota_cols + P), np.float32)
    if CMAJOR:
        iota = np.repeat(np.arange(WIN, dtype=np.float32), max_np)
        cst_host[:, 0:iota_cols] = iota[None, :]
    else:
        cst_host[:, 0:WIN] = np.arange(WIN, dtype=np.float32)[None, :]
    cst_host[:, iota_cols:] = np.eye(P, dtype=np.float32)
    gb_host = np.zeros((P, 2 * D), np.float32)
    gb_host[:, 0:D] = gamma[None, :]
    gb_host[:, D:2 * D] = beta[None, :]
    return wc_host, cst_host, gb_host


def kernel(feat, W0, W1, W2, a0, a1, a2, ln_gamma, ln_beta,
           src0, dst0, src1, dst1, src2, dst2):
    feat = np.asarray(feat, np.float32)
    prep = _host_prep(W0, W1, W2, a0, a1, a2,
                      [src0, src1, src2], [dst0, dst1, dst2])

    gamma = np.asarray(ln_gamma, np.float32).ravel()
    beta = np.asarray(ln_beta, np.float32).ravel()
    apply_affine = not (np.all(gamma == 1.0) and np.all(beta == 0.0))

    nc = _build_nc(prep["schedule"], prep["total_tiles"], prep["total_pairs"],
                   apply_affine)

    tab_bf16 = feat.astype(NP_BF16)
    wc_host, cst_host, gb_host = _host_tensors(prep, gamma, beta)

    in_maps = []
    for c in range(NC):
        in_maps.append({
            "tab_lo": tab_bf16[:LO_SPLIT],
            "tab_hi": tab_bf16[LO_SPLIT:],
            "idx": prep["idx16"][c],
            "dl": _bf16(prep["dlmat"][c]),
            "wg": _bf16(prep["wgmat"][c]),
            "wc": _bf16(wc_host),
            "cst": _bf16(cst_host),
            "gb": gb_host,
        })

    trace = os.environ.get("BENCH_TRACE", "0") == "1"
    kwargs = {}
    if trace:
        tmpdir = os.environ.get("BENCH_TRACE_DIR", "/tmp/kernel_trace")
        os.makedirs(tmpdir, exist_ok=True)
        kwargs = dict(trace=True, tmpdir=tmpdir)
    res = run_bass_kernel_spmd(nc, in_maps, core_ids=list(range(NC)), **kwargs)
    if trace and res.exec_time_ns:
        print(f"HW exec time: {res.exec_time_ns} ns")

    out = np.concatenate([np.asarray(res.results[c]["out"]) for c in range(NC)],
                         axis=0)
    return out[:N].astype(np.float32)
